# revision 11
# baseline (speedup 1.0000x reference)
"""Trainium2 Bass kernel for an 8-layer mirrored-coupling normalizing flow.

Layout strategy: feature-major on chip (features on SBUF partitions, batch on
the free dim) so every MLP matmul uses the weights as-stored as the stationary
operand and activations stream through.  W3 is pre-split on the host into its
s/t column halves so s, t, and the carried state all live on partitions 0:32,
keeping every elementwise op partition-aligned.  PE transpose-mode converts
between HBM batch-major and on-chip feature-major at the edges.

Data parallel over 8 NeuronCores: batch axis sharded, weights replicated.
"""

import sys

import numpy as np

sys.path.insert(0, "/opt/trn_rl_repo")

NCORES = 8
BATCH = 262144
BC = BATCH // NCORES  # 32768 per core
LATENT = 64
HALF = 32
HID = 128
NL = 8
NMLP = 2 * NL  # 16 coupling MLPs
SCALE = 0.1

F = 1024  # batch elements per processed chunk
NCHUNK = BC // F
SUB = F // 128  # 128-row subtiles per chunk for transposes
NMM = F // 512  # matmuls per PSUM tensor fill

_CACHE = {}


def _build_nc(bc=BC):
    import concourse.bass as bass
    import concourse.bacc as bacc
    import concourse.tile as tile
    from concourse import mybir

    fp32 = mybir.dt.float32
    bf16 = mybir.dt.bfloat16
    AF = mybir.ActivationFunctionType
    PSUM = bass.MemorySpace.PSUM
    nchunk = bc // F

    nc = bacc.Bacc(None, target_bir_lowering=False)

    x_d = nc.dram_tensor("x", [bc, LATENT], fp32, kind="ExternalInput")
    w1_d = nc.dram_tensor("w1", [NMLP, HALF, HID], bf16, kind="ExternalInput")
    w2_d = nc.dram_tensor("w2", [NMLP, HID, HID], bf16, kind="ExternalInput")
    w3s_d = nc.dram_tensor("w3s", [NMLP, HID, HALF], bf16, kind="ExternalInput")
    w3t_d = nc.dram_tensor("w3t", [NMLP, HID, HALF], bf16, kind="ExternalInput")
    b1_d = nc.dram_tensor("b1", [HID, NMLP], fp32, kind="ExternalInput")
    b2_d = nc.dram_tensor("b2", [HID, NMLP], fp32, kind="ExternalInput")
    b3s_d = nc.dram_tensor("b3s", [HALF, NMLP], fp32, kind="ExternalInput")
    ident_d = nc.dram_tensor("ident", [128, 128], fp32, kind="ExternalInput")
    y_d = nc.dram_tensor("y", [bc, LATENT], fp32, kind="ExternalOutput")

    with tile.TileContext(nc) as tc:
        with (
            tc.tile_pool(name="weights", bufs=1) as wpool,
            tc.tile_pool(name="io", bufs=3) as iopool,
            tc.tile_pool(name="acts", bufs=3) as actpool,
            tc.tile_pool(name="state", bufs=3) as stpool,
            tc.tile_pool(name="small", bufs=4) as smpool,
            tc.tile_pool(name="hpsum", bufs=1, space=PSUM) as hpsum,
            tc.tile_pool(name="stpsum", bufs=4, space=PSUM) as stpsum,
            tc.tile_pool(name="trpsum", bufs=1, space=PSUM) as trpsum,
        ):
            # ---- resident weights/constants ----
            w1_t = wpool.tile([HALF, NMLP, HID], bf16)
            nc.sync.dma_start(w1_t[:], w1_d.rearrange("k p m -> p k m"))
            w2_t = wpool.tile([HID, NMLP, HID], bf16)
            nc.sync.dma_start(w2_t[:], w2_d.rearrange("k p m -> p k m"))
            w3s_t = wpool.tile([HID, NMLP, HALF], bf16)
            nc.sync.dma_start(w3s_t[:], w3s_d.rearrange("k p m -> p k m"))
            w3t_t = wpool.tile([HID, NMLP, HALF], bf16)
            nc.sync.dma_start(w3t_t[:], w3t_d.rearrange("k p m -> p k m"))
            b1_t = wpool.tile([HID, NMLP], fp32)
            nc.sync.dma_start(b1_t[:], b1_d[:])
            b2_t = wpool.tile([HID, NMLP], fp32)
            nc.sync.dma_start(b2_t[:], b2_d[:])
            b3s_t = wpool.tile([HALF, NMLP], fp32)
            nc.sync.dma_start(b3s_t[:], b3s_d[:])
            ident_t = wpool.tile([128, 128], fp32)
            nc.sync.dma_start(ident_t[:], ident_d[:])

            for g in range(nchunk):
                xrows = x_d[g * F : (g + 1) * F, :]
                stage = iopool.tile([128, SUB, LATENT], fp32, tag="stage")
                nc.sync.dma_start(
                    stage[:], xrows.rearrange("(s p) f -> p s f", p=128)
                )
                trp = trpsum.tile([LATENT, F], fp32, tag="tr")
                for s in range(SUB):
                    nc.tensor.transpose(
                        trp[:, s * 128 : (s + 1) * 128],
                        stage[:, s, :],
                        ident_t[:],
                    )
                trsb = iopool.tile([LATENT, F], fp32, tag="trsb")
                nc.scalar.activation(trsb[:], trp[:], AF.Copy)

                A = stpool.tile([HALF, F], fp32, tag="A")
                nc.sync.dma_start(A[:], trsb[0:HALF, :])
                B = stpool.tile([HALF, F], fp32, tag="B")
                nc.sync.dma_start(B[:], trsb[HALF:LATENT, :])
                Ab = stpool.tile([HALF, F], bf16, tag="Ab")
                nc.vector.tensor_copy(Ab[:], A[:])
                Bb = stpool.tile([HALF, F], bf16, tag="Bb")
                nc.vector.tensor_copy(Bb[:], B[:])

                for l in range(NL):
                    for side in range(2):
                        li = 2 * l + side
                        inp = Ab if side == 0 else Bb  # conditions the MLP
                        mst = B if side == 0 else A  # gets transformed

                        h1 = hpsum.tile([HID, F], fp32, tag="h")
                        for j in range(NMM):
                            jj = slice(j * 512, (j + 1) * 512)
                            nc.tensor.matmul(
                                h1[:, jj], w1_t[:, li, :], inp[:, jj]
                            )
                        h1s = actpool.tile([HID, F], bf16, tag="h1s")
                        nc.scalar.activation(
                            h1s[:], h1[:], AF.Relu, bias=b1_t[:, li : li + 1]
                        )

                        h2 = hpsum.tile([HID, F], fp32, tag="h")
                        for j in range(NMM):
                            jj = slice(j * 512, (j + 1) * 512)
                            nc.tensor.matmul(
                                h2[:, jj], w2_t[:, li, :], h1s[:, jj]
                            )
                        h2s = actpool.tile([HID, F], bf16, tag="h2s")
                        nc.scalar.activation(
                            h2s[:], h2[:], AF.Relu, bias=b2_t[:, li : li + 1]
                        )

                        sp = stpsum.tile([HALF, 512], fp32, tag="st")
                        tp = stpsum.tile([HALF, 512], fp32, tag="st")
                        sp2 = stpsum.tile([HALF, 512], fp32, tag="st")
                        tp2 = stpsum.tile([HALF, 512], fp32, tag="st")
                        sps = [sp, sp2]
                        tps = [tp, tp2]
                        for j in range(NMM):
                            jj = slice(j * 512, (j + 1) * 512)
                            nc.tensor.matmul(
                                sps[j][:], w3s_t[:, li, :], h2s[:, jj]
                            )
                            nc.tensor.matmul(
                                tps[j][:], w3t_t[:, li, :], h2s[:, jj]
                            )

                        mnew = stpool.tile([HALF, F], fp32, tag="B" if side == 0 else "A")
                        mnewb = stpool.tile([HALF, F], bf16, tag="Bb" if side == 0 else "Ab")
                        for j in range(NMM):
                            jj = slice(j * 512, (j + 1) * 512)
                            u = smpool.tile([HALF, 512], fp32, tag="u")
                            nc.scalar.activation(
                                u[:], sps[j][:], AF.Tanh, bias=b3s_t[:, li : li + 1]
                            )
                            e = smpool.tile([HALF, 512], fp32, tag="e")
                            nc.scalar.activation(e[:], u[:], AF.Exp, scale=SCALE)
                            tmp = smpool.tile([HALF, 512], fp32, tag="tmp")
                            nc.vector.tensor_mul(tmp[:], mst[:, jj], e[:])
                            nc.vector.tensor_add(mnew[:, jj], tmp[:], tps[j][:])
                            nc.vector.tensor_copy(mnewb[:, jj], mnew[:, jj])

                        if side == 0:
                            B, Bb = mnew, mnewb
                        else:
                            A, Ab = mnew, mnewb

                # ---- output edge ----
                osb = iopool.tile([LATENT, F], fp32, tag="osb")
                nc.sync.dma_start(osb[0:HALF, :], A[:])
                nc.sync.dma_start(osb[HALF:LATENT, :], B[:])
                otp = trpsum.tile([128, F // 2], fp32, tag="tr")
                for s in range(SUB):
                    nc.tensor.transpose(
                        otp[:, s * LATENT : (s + 1) * LATENT],
                        osb[:, s * 128 : (s + 1) * 128],
                        ident_t[0:LATENT, 0:LATENT],
                    )
                ost = iopool.tile([128, SUB, LATENT], fp32, tag="ost")
                nc.scalar.activation(
                    ost[:], otp[:].rearrange("p (s f) -> p s f", s=SUB), AF.Copy
                )
                yrows = y_d[g * F : (g + 1) * F, :]
                nc.sync.dma_start(
                    yrows.rearrange("(s p) f -> p s f", p=128), ost[:]
                )

    nc.compile()
    return nc


def _get_nc(bc=BC):
    key = ("nc", bc)
    if key not in _CACHE:
        _CACHE[key] = _build_nc(bc)
    return _CACHE[key]


def _prep_inputs(x, W1, b1, W2, b2, W3, b3):
    import ml_dtypes

    bf16 = ml_dtypes.bfloat16
    # [NL, 2, ...] -> [NMLP, ...] with li = 2*l + side
    W1f = np.ascontiguousarray(W1.reshape(NMLP, HALF, HID).astype(bf16))
    W2f = np.ascontiguousarray(W2.reshape(NMLP, HID, HID).astype(bf16))
    W3f = W3.reshape(NMLP, HID, LATENT)
    W3s = np.ascontiguousarray(W3f[:, :, :HALF].astype(bf16))
    W3t = np.ascontiguousarray(W3f[:, :, HALF:].astype(bf16))
    b1f = np.ascontiguousarray(b1.reshape(NMLP, HID).T.astype(np.float32))
    b2f = np.ascontiguousarray(b2.reshape(NMLP, HID).T.astype(np.float32))
    b3f = b3.reshape(NMLP, LATENT)
    b3s = np.ascontiguousarray(b3f[:, :HALF].T.astype(np.float32))
    b3t = b3f[:, HALF:]
    assert not np.any(b3t), "nonzero b3 t-half not supported by this kernel build"
    ident = np.eye(128, dtype=np.float32)

    shared = {
        "w1": W1f,
        "w2": W2f,
        "w3s": W3s,
        "w3t": W3t,
        "b1": b1f,
        "b2": b2f,
        "b3s": b3s,
        "ident": ident,
    }
    x = np.ascontiguousarray(x.astype(np.float32))
    in_maps = []
    for c in range(NCORES):
        m = dict(shared)
        m["x"] = x[c * BC : (c + 1) * BC]
        in_maps.append(m)
    return in_maps


def _ensure_ntff_hook():
    """The agent image lacks ``antenv.axon_hooks``; shim it so trace=True works."""
    import sys as _sys
    import types

    try:
        from antenv import axon_hooks  # noqa: F401
        return
    except ImportError:
        pass
    import antenv
    from trn_agent_boot.trn_boot import _ntff_profile_via_ctypes

    hook = _ntff_profile_via_ctypes("/opt/axon/libaxon_pjrt.so")
    mod = types.ModuleType("antenv.axon_hooks")
    mod._hook = hook
    mod.get_axon_ntff_profile_hook = lambda: mod._hook
    mod.set_axon_ntff_profile_hook = lambda h: setattr(mod, "_hook", h)
    _sys.modules["antenv.axon_hooks"] = mod
    antenv.axon_hooks = mod


def _patch_upload():
    """Uploading artifacts to the bucket may fail in this sandbox; degrade to local."""
    from concourse import bass_utils

    orig = bass_utils.upload_artifacts
    if getattr(orig, "_patched", False):
        return

    def safe_upload(tmpdir):
        try:
            return orig(tmpdir)
        except Exception as e:
            return f"{tmpdir} (upload skipped: {type(e).__name__})"

    safe_upload._patched = True
    bass_utils.upload_artifacts = safe_upload


def run(inputs, trace=False, **kw):
    from concourse.bass_utils import run_bass_kernel_spmd

    if trace:
        _ensure_ntff_hook()
        _patch_upload()
    nc = _get_nc()
    in_maps = _prep_inputs(
        inputs["x"], inputs["W1"], inputs["b1"], inputs["W2"],
        inputs["b2"], inputs["W3"], inputs["b3"],
    )
    res = run_bass_kernel_spmd(nc, in_maps, list(range(NCORES)), trace=trace, **kw)
    y = np.concatenate([res.results[c]["y"] for c in range(NCORES)], axis=0)
    return y, res


def kernel(**inputs):
    y, _ = run(inputs, trace=False)
    return y


# revision 12
# speedup vs baseline: 1.4931x; 1.4931x over previous
"""v2 (bf16 matmuls): 4-chunk packed groups, all-fp32, PE row/col tiling, ACT/DVE split.

Packing: 4 chunks of F=512 batch elements are stacked on SBUF partition
strips (32 rows each) for the coupling half-width tensors, so tanh/exp/
mul/add run at full 128-partition width.  mm1 runs as 4 concurrent row
tiles (K=32), mm3 as 4 concurrent column tiles (M=32) twice (s-half and
t-half of W3), packing s and t for all 4 chunks densely into single PSUM
banks that are partition-aligned with the packed state.
"""

import sys

import numpy as np

sys.path.insert(0, "/opt/trn_rl_repo")

NCORES = 8
BATCH = 262144
BC = BATCH // NCORES
LATENT = 64
HALF = 32
HID = 128
NL = 8
NMLP = 2 * NL
SCALE = 0.1

F = 512              # batch elements per chunk (= PSUM bank free size)
G = 4                # chunks packed per group
GE = G * F           # 2048 elements per group
SUB = F // 128       # 4 transpose subtiles per chunk

_CACHE = {}


def _build_nc(bc=BC):
    import concourse.bass as bass
    import concourse.bacc as bacc
    import concourse.tile as tile
    from concourse import mybir

    fp32 = mybir.dt.float32
    bf16 = mybir.dt.bfloat16
    AF = mybir.ActivationFunctionType
    ALU = mybir.AluOpType
    PSUM = bass.MemorySpace.PSUM
    ngroup = bc // GE

    nc = bacc.Bacc(None, target_bir_lowering=False)

    x_d = nc.dram_tensor("x", [bc, LATENT], fp32, kind="ExternalInput")
    w1r_d = nc.dram_tensor("w1r", [NMLP, 128, HID], bf16, kind="ExternalInput")
    w2_d = nc.dram_tensor("w2", [NMLP, HID, HID], bf16, kind="ExternalInput")
    w3s_d = nc.dram_tensor("w3s", [NMLP, HID, HALF], bf16, kind="ExternalInput")
    w3t_d = nc.dram_tensor("w3t", [NMLP, HID, HALF], bf16, kind="ExternalInput")
    b1_d = nc.dram_tensor("b1", [HID, NMLP], fp32, kind="ExternalInput")
    b2_d = nc.dram_tensor("b2", [HID, NMLP], fp32, kind="ExternalInput")
    b3sr_d = nc.dram_tensor("b3sr", [128, NMLP], fp32, kind="ExternalInput")
    ident_d = nc.dram_tensor("ident", [128, 128], fp32, kind="ExternalInput")
    y_d = nc.dram_tensor("y", [bc, LATENT], fp32, kind="ExternalOutput")

    with tile.TileContext(nc) as tc:
        with (
            tc.tile_pool(name="weights", bufs=1) as wpool,
            tc.tile_pool(name="io", bufs=4) as iopool,
            tc.tile_pool(name="acts", bufs=3) as actpool,
            tc.tile_pool(name="state", bufs=3) as stpool,
            tc.tile_pool(name="small", bufs=4) as smpool,
            tc.tile_pool(name="hpsum", bufs=3, space=PSUM) as hpsum,
            tc.tile_pool(name="stpsum", bufs=2, space=PSUM) as stpsum,
        ):
            # ---- resident weights/constants ----
            w1r_t = wpool.tile([128, NMLP, HID], bf16)
            nc.sync.dma_start(w1r_t[:], w1r_d.rearrange("k p m -> p k m"))
            w2_t = wpool.tile([HID, NMLP, HID], bf16)
            nc.sync.dma_start(w2_t[:], w2_d.rearrange("k p m -> p k m"))
            w3s_t = wpool.tile([HID, NMLP, HALF], bf16)
            nc.sync.dma_start(w3s_t[:], w3s_d.rearrange("k p m -> p k m"))
            w3t_t = wpool.tile([HID, NMLP, HALF], bf16)
            nc.sync.dma_start(w3t_t[:], w3t_d.rearrange("k p m -> p k m"))
            b1_t = wpool.tile([HID, NMLP], fp32)
            nc.sync.dma_start(b1_t[:], b1_d[:])
            b2_t = wpool.tile([HID, NMLP], fp32)
            nc.sync.dma_start(b2_t[:], b2_d[:])
            b3sr_t = wpool.tile([128, NMLP], fp32)
            nc.sync.dma_start(b3sr_t[:], b3sr_d[:])
            ident_t = wpool.tile([128, 128], fp32)
            nc.sync.dma_start(ident_t[:], ident_d[:])

            for g in range(ngroup):
                # ---- input edge: 4 chunks -> packed X1/X2 [128, F] ----
                X1 = stpool.tile([128, F], fp32, tag="X1")
                X2 = stpool.tile([128, F], fp32, tag="X2")
                X1b = stpool.tile([128, F], bf16, tag="X1b")
                X2b = stpool.tile([128, F], bf16, tag="X2b")
                for c in range(G):
                    base = g * GE + c * F
                    xrows = x_d[base : base + F, :]
                    stage = iopool.tile([128, SUB, LATENT], fp32, tag="stage")
                    nc.sync.dma_start(
                        stage[:], xrows.rearrange("(s p) f -> p s f", p=128)
                    )
                    trp = stpsum.tile([128, F // 2], fp32, tag="st")
                    for s in range(SUB // 2):
                        nc.tensor.transpose(
                            trp[:, s * 128 : (s + 1) * 128],
                            stage[:, 2 * s : 2 * s + 2, :].rearrange(
                                "p s f -> p (s f)"
                            ),
                            ident_t[:],
                        )
                    trsb = iopool.tile([128, F // 2], fp32, tag="trsb")
                    if c % 2 == 0:
                        nc.scalar.activation(trsb[:], trp[:], AF.Copy)
                    else:
                        nc.vector.tensor_copy(trsb[:], trp[:])
                    # trsb rows: 0:32 x1(even sub), 32:64 x2(even), 64:96 x1(odd), 96:128 x2(odd)
                    for h in range(2):
                        x1v = X1[c * HALF : (c + 1) * HALF, :].rearrange(
                            "q (s hh f) -> q s hh f", hh=2, f=128
                        )
                        nc.sync.dma_start(
                            x1v[:, :, h, :],
                            trsb[64 * h : 64 * h + HALF, :].rearrange(
                                "q (s f) -> q s f", f=128
                            ),
                        )
                        x2v = X2[c * HALF : (c + 1) * HALF, :].rearrange(
                            "q (s hh f) -> q s hh f", hh=2, f=128
                        )
                        nc.sync.dma_start(
                            x2v[:, :, h, :],
                            trsb[64 * h + HALF : 64 * (h + 1), :].rearrange(
                                "q (s f) -> q s f", f=128
                            ),
                        )

                nc.vector.tensor_copy(X1b[:], X1[:])
                nc.vector.tensor_copy(X2b[:], X2[:])

                # ---- 16 coupling MLPs ----
                for li in range(NMLP):
                    side = li % 2
                    inp = X1b if side == 0 else X2b
                    mst = X2 if side == 0 else X1

                    # mm1: 4 concurrent row tiles (K=32), pair PSUM tensors
                    h1 = [
                        hpsum.tile([HID, 2 * F], fp32, tag="h", name="h1a"),
                        hpsum.tile([HID, 2 * F], fp32, tag="h", name="h1b"),
                    ]
                    for c in range(G):
                        nc.tensor.matmul(
                            h1[c // 2][:, (c % 2) * F : (c % 2 + 1) * F],
                            w1r_t[c * HALF : (c + 1) * HALF, li, :],
                            inp[c * HALF : (c + 1) * HALF, :],
                            tile_position=(c * HALF, 0),
                        )
                    r1 = [
                        actpool.tile([HID, 2 * F], bf16, tag="r1", name="r1a"),
                        actpool.tile([HID, 2 * F], bf16, tag="r1", name="r1b"),
                    ]
                    nc.scalar.activation(
                        r1[0][:], h1[0][:], AF.Relu, bias=b1_t[:, li : li + 1]
                    )
                    nc.vector.tensor_scalar(
                        r1[1][:], h1[1][:], b1_t[:, li : li + 1], 0.0,
                        ALU.add, ALU.max,
                    )

                    # mm2: full-array K=128
                    h2 = [
                        hpsum.tile([HID, 2 * F], fp32, tag="h", name="h2a"),
                        hpsum.tile([HID, 2 * F], fp32, tag="h", name="h2b"),
                    ]
                    for c in range(G):
                        nc.tensor.matmul(
                            h2[c // 2][:, (c % 2) * F : (c % 2 + 1) * F],
                            w2_t[:, li, :],
                            r1[c // 2][:, (c % 2) * F : (c % 2 + 1) * F],
                        )
                    r2 = [
                        actpool.tile([HID, 2 * F], bf16, tag="r2", name="r2a"),
                        actpool.tile([HID, 2 * F], bf16, tag="r2", name="r2b"),
                    ]
                    nc.scalar.activation(
                        r2[0][:], h2[0][:], AF.Relu, bias=b2_t[:, li : li + 1]
                    )
                    nc.vector.tensor_scalar(
                        r2[1][:], h2[1][:], b2_t[:, li : li + 1], 0.0,
                        ALU.add, ALU.max,
                    )

                    # mm3: 4 concurrent column tiles (M=32) x {s, t}
                    sbank = stpsum.tile([128, F], fp32, tag="st")
                    tbank = stpsum.tile([128, F], fp32, tag="st")
                    for c in range(G):
                        rhs = r2[c // 2][:, (c % 2) * F : (c % 2 + 1) * F]
                        nc.tensor.matmul(
                            sbank[c * HALF : (c + 1) * HALF, :],
                            w3s_t[:, li, :],
                            rhs,
                            tile_position=(0, c * HALF),
                        )
                    for c in range(G):
                        rhs = r2[c // 2][:, (c % 2) * F : (c % 2 + 1) * F]
                        nc.tensor.matmul(
                            tbank[c * HALF : (c + 1) * HALF, :],
                            w3t_t[:, li, :],
                            rhs,
                            tile_position=(0, c * HALF),
                        )

                    u = smpool.tile([128, F], fp32, tag="u")
                    nc.scalar.activation(
                        u[:], sbank[:], AF.Tanh, bias=b3sr_t[:, li : li + 1]
                    )
                    e = smpool.tile([128, F], fp32, tag="e")
                    nc.scalar.activation(e[:], u[:], AF.Exp, scale=SCALE)
                    tmp = smpool.tile([128, F], fp32, tag="tmp")
                    nc.gpsimd.tensor_mul(tmp[:], mst[:], e[:])
                    newm = stpool.tile(
                        [128, F], fp32, tag="X2" if side == 0 else "X1"
                    )
                    nc.vector.tensor_add(newm[:], tmp[:], tbank[:])
                    newmb = stpool.tile(
                        [128, F], bf16, tag="X2b" if side == 0 else "X1b"
                    )
                    nc.vector.tensor_copy(newmb[:], newm[:])

                    if side == 0:
                        X2, X2b = newm, newmb
                    else:
                        X1, X1b = newm, newmb

                # ---- output edge ----
                for c in range(G):
                    osb = iopool.tile([LATENT, F], fp32, tag="osb")
                    nc.sync.dma_start(
                        osb[0:HALF, :], X1[c * HALF : (c + 1) * HALF, :]
                    )
                    nc.sync.dma_start(
                        osb[HALF:LATENT, :], X2[c * HALF : (c + 1) * HALF, :]
                    )
                    otp = stpsum.tile([128, SUB * LATENT], fp32, tag="st")
                    for s in range(SUB):
                        nc.tensor.transpose(
                            otp[:, s * LATENT : (s + 1) * LATENT],
                            osb[:, s * 128 : (s + 1) * 128],
                            ident_t[0:LATENT, 0:LATENT],
                        )
                    ost = iopool.tile([128, SUB, LATENT], fp32, tag="ost")
                    if c % 2 == 0:
                        nc.scalar.activation(
                            ost[:],
                            otp[:].rearrange("p (s f) -> p s f", s=SUB),
                            AF.Copy,
                        )
                    else:
                        nc.vector.tensor_copy(
                            ost[:],
                            otp[:].rearrange("p (s f) -> p s f", s=SUB),
                        )
                    base = g * GE + c * F
                    yrows = y_d[base : base + F, :]
                    nc.sync.dma_start(
                        yrows.rearrange("(s p) f -> p s f", p=128), ost[:]
                    )

    nc.compile()
    return nc


def _get_nc(bc=BC):
    key = ("nc", bc)
    if key not in _CACHE:
        _CACHE[key] = _build_nc(bc)
    return _CACHE[key]


def _prep_inputs(x, W1, b1, W2, b2, W3, b3):
    import ml_dtypes

    f32 = np.float32
    bf = ml_dtypes.bfloat16
    W1f = W1.reshape(NMLP, HALF, HID).astype(bf)
    W1r = np.ascontiguousarray(np.tile(W1f, (1, 4, 1)))  # [NMLP, 128, HID]
    W2f = np.ascontiguousarray(W2.reshape(NMLP, HID, HID).astype(bf))
    W3f = W3.reshape(NMLP, HID, LATENT).astype(bf)
    W3s = np.ascontiguousarray(W3f[:, :, :HALF])
    W3t = np.ascontiguousarray(W3f[:, :, HALF:])
    b1f = np.ascontiguousarray(b1.reshape(NMLP, HID).T.astype(f32))
    b2f = np.ascontiguousarray(b2.reshape(NMLP, HID).T.astype(f32))
    b3f = b3.reshape(NMLP, LATENT).astype(f32)
    b3sr = np.ascontiguousarray(np.tile(b3f[:, :HALF], (1, 4)).T)  # [128, NMLP]
    b3t = b3f[:, HALF:]
    assert not np.any(b3t), "nonzero b3 t-half not supported by this kernel build"
    ident = np.eye(128, dtype=f32)

    shared = {
        "w1r": W1r,
        "w2": W2f,
        "w3s": W3s,
        "w3t": W3t,
        "b1": b1f,
        "b2": b2f,
        "b3sr": b3sr,
        "ident": ident,
    }
    x = np.ascontiguousarray(x.astype(f32))
    in_maps = []
    for c in range(NCORES):
        m = dict(shared)
        m["x"] = x[c * BC : (c + 1) * BC]
        in_maps.append(m)
    return in_maps


def _ensure_ntff_hook():
    import sys as _sys
    import types

    try:
        from antenv import axon_hooks  # noqa: F401
        return
    except ImportError:
        pass
    import antenv
    from trn_agent_boot.trn_boot import _ntff_profile_via_ctypes

    hook = _ntff_profile_via_ctypes("/opt/axon/libaxon_pjrt.so")
    mod = types.ModuleType("antenv.axon_hooks")
    mod._hook = hook
    mod.get_axon_ntff_profile_hook = lambda: mod._hook
    mod.set_axon_ntff_profile_hook = lambda h: setattr(mod, "_hook", h)
    _sys.modules["antenv.axon_hooks"] = mod
    antenv.axon_hooks = mod


def _patch_upload():
    from concourse import bass_utils

    orig = bass_utils.upload_artifacts
    if getattr(orig, "_patched", False):
        return

    def safe_upload(tmpdir):
        try:
            return orig(tmpdir)
        except Exception as e:
            return f"{tmpdir} (upload skipped: {type(e).__name__})"

    safe_upload._patched = True
    bass_utils.upload_artifacts = safe_upload


def run(inputs, trace=False, **kw):
    from concourse.bass_utils import run_bass_kernel_spmd

    if trace:
        _ensure_ntff_hook()
        _patch_upload()
    nc = _get_nc()
    in_maps = _prep_inputs(
        inputs["x"], inputs["W1"], inputs["b1"], inputs["W2"],
        inputs["b2"], inputs["W3"], inputs["b3"],
    )
    res = run_bass_kernel_spmd(nc, in_maps, list(range(NCORES)), trace=trace, **kw)
    y = np.concatenate([res.results[c]["y"] for c in range(NCORES)], axis=0)
    return y, res


def kernel(**inputs):
    y, _ = run(inputs, trace=False)
    return y


# revision 13
# speedup vs baseline: 1.5525x; 1.0398x over previous
"""v4 (bf16 mm1 + shadows): two-group interleaved emission, fp32 mm1 (no state casts), bf16 mm2/mm3.

Each coupling MLP has a long serial dependency chain (mm1 -> relu -> mm2 ->
relu -> mm3 -> tanh -> exp -> mul -> add); one group alone leaves every
engine half idle.  v3 interleaves two independent batch groups instruction-
by-instruction so ACT/DVE/PE always have a second chain to work on.

mm1 takes the fp32 packed state directly (fp32 matmul = 2 PE passes, PE has
headroom), which removes every bf16 state-shadow cast from the chain.  mm2
and mm3 read the relus' bf16 outputs (the PSUM->SBUF relu pass casts for
free).
"""

import sys

import numpy as np

sys.path.insert(0, "/opt/trn_rl_repo")

NCORES = 8
BATCH = 262144
BC = BATCH // NCORES
LATENT = 64
HALF = 32
HID = 128
NL = 8
NMLP = 2 * NL
SCALE = 0.1

F = 512              # batch elements per chunk (= PSUM bank free size)
G = 4                # chunks packed per group
GE = G * F           # 2048 elements per group
SUB = F // 128       # transpose subtiles per chunk
NGIL = 2             # groups interleaved

_CACHE = {}


def _build_nc(bc=BC):
    import concourse.bass as bass
    import concourse.bacc as bacc
    import concourse.tile as tile
    from concourse import mybir

    fp32 = mybir.dt.float32
    bf16 = mybir.dt.bfloat16
    AF = mybir.ActivationFunctionType
    ALU = mybir.AluOpType
    PSUM = bass.MemorySpace.PSUM
    ngroup = bc // GE
    assert ngroup % NGIL == 0

    nc = bacc.Bacc(None, target_bir_lowering=False)

    x_d = nc.dram_tensor("x", [bc, LATENT], fp32, kind="ExternalInput")
    w1r_d = nc.dram_tensor("w1r", [NMLP, 128, HID], bf16, kind="ExternalInput")
    w2_d = nc.dram_tensor("w2", [NMLP, HID, HID], bf16, kind="ExternalInput")
    w3s_d = nc.dram_tensor("w3s", [NMLP, HID, HALF], bf16, kind="ExternalInput")
    w3t_d = nc.dram_tensor("w3t", [NMLP, HID, HALF], bf16, kind="ExternalInput")
    b1_d = nc.dram_tensor("b1", [HID, NMLP], fp32, kind="ExternalInput")
    b2_d = nc.dram_tensor("b2", [HID, NMLP], fp32, kind="ExternalInput")
    b3sr_d = nc.dram_tensor("b3sr", [128, NMLP], fp32, kind="ExternalInput")
    ident_d = nc.dram_tensor("ident", [128, 128], fp32, kind="ExternalInput")
    y_d = nc.dram_tensor("y", [bc, LATENT], fp32, kind="ExternalOutput")

    with tile.TileContext(nc) as tc:
        with (
            tc.tile_pool(name="weights", bufs=1) as wpool,
            tc.tile_pool(name="io", bufs=6) as iopool,
            tc.tile_pool(name="acts", bufs=4) as actpool,
            tc.tile_pool(name="state", bufs=4) as stpool,
            tc.tile_pool(name="small", bufs=6) as smpool,
            tc.tile_pool(name="hpsum", bufs=2, space=PSUM) as hpsum,
            tc.tile_pool(name="stpsum", bufs=4, space=PSUM) as stpsum,
        ):
            w1r_t = wpool.tile([128, NMLP, HID], bf16)
            nc.sync.dma_start(w1r_t[:], w1r_d.rearrange("k p m -> p k m"))
            w2_t = wpool.tile([HID, NMLP, HID], bf16)
            nc.sync.dma_start(w2_t[:], w2_d.rearrange("k p m -> p k m"))
            w3s_t = wpool.tile([HID, NMLP, HALF], bf16)
            nc.sync.dma_start(w3s_t[:], w3s_d.rearrange("k p m -> p k m"))
            w3t_t = wpool.tile([HID, NMLP, HALF], bf16)
            nc.sync.dma_start(w3t_t[:], w3t_d.rearrange("k p m -> p k m"))
            b1_t = wpool.tile([HID, NMLP], fp32)
            nc.sync.dma_start(b1_t[:], b1_d[:])
            b2_t = wpool.tile([HID, NMLP], fp32)
            nc.sync.dma_start(b2_t[:], b2_d[:])
            b3sr_t = wpool.tile([128, NMLP], fp32)
            nc.sync.dma_start(b3sr_t[:], b3sr_d[:])
            ident_t = wpool.tile([128, 128], fp32)
            nc.sync.dma_start(ident_t[:], ident_d[:])

            def in_edge(g, k):
                X1 = stpool.tile([128, F], fp32, tag=f"X1_{k}", name="X1")
                X2 = stpool.tile([128, F], fp32, tag=f"X2_{k}", name="X2")
                X1b = stpool.tile([128, F], bf16, tag=f"X1b_{k}", name="X1b")
                X2b = stpool.tile([128, F], bf16, tag=f"X2b_{k}", name="X2b")
                for c in range(G):
                    base = g * GE + c * F
                    xrows = x_d[base : base + F, :]
                    stage = iopool.tile(
                        [128, SUB, LATENT], fp32, tag="stage", name="stage"
                    )
                    nc.sync.dma_start(
                        stage[:], xrows.rearrange("(s p) f -> p s f", p=128)
                    )
                    trp = stpsum.tile([128, F // 2], fp32, tag="st", name="trp")
                    for s in range(SUB // 2):
                        nc.tensor.transpose(
                            trp[:, s * 128 : (s + 1) * 128],
                            stage[:, 2 * s : 2 * s + 2, :].rearrange(
                                "p s f -> p (s f)"
                            ),
                            ident_t[:],
                        )
                    trsb = iopool.tile([128, F // 2], fp32, tag="trsb", name="trsb")
                    if c % 2 == 0:
                        nc.scalar.activation(trsb[:], trp[:], AF.Copy)
                    else:
                        nc.vector.tensor_copy(trsb[:], trp[:])
                    # trsb rows: 0:32 x1(even sub), 32:64 x2(even), 64:96 x1(odd), 96:128 x2(odd)
                    for h in range(2):
                        x1v = X1[c * HALF : (c + 1) * HALF, :].rearrange(
                            "q (s hh f) -> q s hh f", hh=2, f=128
                        )
                        nc.sync.dma_start(
                            x1v[:, :, h, :],
                            trsb[64 * h : 64 * h + HALF, :].rearrange(
                                "q (s f) -> q s f", f=128
                            ),
                        )
                        x2v = X2[c * HALF : (c + 1) * HALF, :].rearrange(
                            "q (s hh f) -> q s hh f", hh=2, f=128
                        )
                        nc.sync.dma_start(
                            x2v[:, :, h, :],
                            trsb[64 * h + HALF : 64 * (h + 1), :].rearrange(
                                "q (s f) -> q s f", f=128
                            ),
                        )
                nc.vector.tensor_copy(X1b[:], X1[:])
                nc.vector.tensor_copy(X2b[:], X2[:])
                return X1, X2, X1b, X2b

            def mlp(st, li, k):
                side = li % 2
                inp = st["X1b"] if side == 0 else st["X2b"]
                mst = st["X2"] if side == 0 else st["X1"]

                h1 = [
                    hpsum.tile([HID, 2 * F], fp32, tag="h", name="h1a"),
                    hpsum.tile([HID, 2 * F], fp32, tag="h", name="h1b"),
                ]
                for c in range(G):
                    nc.tensor.matmul(
                        h1[c // 2][:, (c % 2) * F : (c % 2 + 1) * F],
                        w1r_t[c * HALF : (c + 1) * HALF, li, :],
                        inp[c * HALF : (c + 1) * HALF, :],
                        tile_position=(c * HALF, 0),
                    )
                r1 = [
                    actpool.tile([HID, 2 * F], bf16, tag="r1", name="r1a"),
                    actpool.tile([HID, 2 * F], bf16, tag="r1", name="r1b"),
                ]
                nc.scalar.activation(
                    r1[0][:], h1[0][:], AF.Relu, bias=b1_t[:, li : li + 1]
                )
                nc.vector.tensor_scalar(
                    r1[1][:], h1[1][:], b1_t[:, li : li + 1], 0.0, ALU.add, ALU.max
                )

                h2 = [
                    hpsum.tile([HID, 2 * F], fp32, tag="h", name="h2a"),
                    hpsum.tile([HID, 2 * F], fp32, tag="h", name="h2b"),
                ]
                for c in range(G):
                    nc.tensor.matmul(
                        h2[c // 2][:, (c % 2) * F : (c % 2 + 1) * F],
                        w2_t[:, li, :],
                        r1[c // 2][:, (c % 2) * F : (c % 2 + 1) * F],
                    )
                r2 = [
                    actpool.tile([HID, 2 * F], bf16, tag="r2", name="r2a"),
                    actpool.tile([HID, 2 * F], bf16, tag="r2", name="r2b"),
                ]
                nc.scalar.activation(
                    r2[0][:], h2[0][:], AF.Relu, bias=b2_t[:, li : li + 1]
                )
                nc.vector.tensor_scalar(
                    r2[1][:], h2[1][:], b2_t[:, li : li + 1], 0.0, ALU.add, ALU.max
                )

                sbank = stpsum.tile([128, F], fp32, tag="st", name="sbank")
                tbank = stpsum.tile([128, F], fp32, tag="st", name="tbank")
                for c in range(G):
                    rhs = r2[c // 2][:, (c % 2) * F : (c % 2 + 1) * F]
                    nc.tensor.matmul(
                        sbank[c * HALF : (c + 1) * HALF, :],
                        w3s_t[:, li, :],
                        rhs,
                        tile_position=(0, c * HALF),
                    )
                for c in range(G):
                    rhs = r2[c // 2][:, (c % 2) * F : (c % 2 + 1) * F]
                    nc.tensor.matmul(
                        tbank[c * HALF : (c + 1) * HALF, :],
                        w3t_t[:, li, :],
                        rhs,
                        tile_position=(0, c * HALF),
                    )

                u = smpool.tile([128, F], fp32, tag="u", name="u")
                nc.scalar.activation(
                    u[:], sbank[:], AF.Tanh, bias=b3sr_t[:, li : li + 1]
                )
                e = smpool.tile([128, F], fp32, tag="e", name="e")
                nc.scalar.activation(e[:], u[:], AF.Exp, scale=SCALE)
                tmp = smpool.tile([128, F], fp32, tag="tmp", name="tmp")
                nc.vector.tensor_mul(tmp[:], mst[:], e[:])
                newm = stpool.tile(
                    [128, F], fp32,
                    tag=(f"X2_{k}" if side == 0 else f"X1_{k}"), name="newm",
                )
                nc.vector.tensor_add(newm[:], tmp[:], tbank[:])
                newmb = stpool.tile(
                    [128, F], bf16,
                    tag=(f"X2b_{k}" if side == 0 else f"X1b_{k}"), name="newmb",
                )
                if li % 2 == 0:
                    nc.scalar.activation(newmb[:], newm[:], AF.Copy)
                else:
                    nc.vector.tensor_copy(newmb[:], newm[:])

                if side == 0:
                    st["X2"], st["X2b"] = newm, newmb
                else:
                    st["X1"], st["X1b"] = newm, newmb

            def out_edge(g, st):
                X1, X2 = st["X1"], st["X2"]
                for c in range(G):
                    osb = iopool.tile([LATENT, F], fp32, tag="osb", name="osb")
                    nc.sync.dma_start(
                        osb[0:HALF, :], X1[c * HALF : (c + 1) * HALF, :]
                    )
                    nc.sync.dma_start(
                        osb[HALF:LATENT, :], X2[c * HALF : (c + 1) * HALF, :]
                    )
                    otp = stpsum.tile(
                        [128, SUB * LATENT], fp32, tag="st", name="otp"
                    )
                    for s in range(SUB):
                        nc.tensor.transpose(
                            otp[:, s * LATENT : (s + 1) * LATENT],
                            osb[:, s * 128 : (s + 1) * 128],
                            ident_t[0:LATENT, 0:LATENT],
                        )
                    ost = iopool.tile([128, SUB, LATENT], fp32, tag="ost", name="ost")
                    if c % 2 == 0:
                        nc.scalar.activation(
                            ost[:],
                            otp[:].rearrange("p (s f) -> p s f", s=SUB),
                            AF.Copy,
                        )
                    else:
                        nc.vector.tensor_copy(
                            ost[:], otp[:].rearrange("p (s f) -> p s f", s=SUB)
                        )
                    base = g * GE + c * F
                    yrows = y_d[base : base + F, :]
                    nc.sync.dma_start(
                        yrows.rearrange("(s p) f -> p s f", p=128), ost[:]
                    )

            for gp in range(ngroup // NGIL):
                gs = [gp * NGIL + k for k in range(NGIL)]
                sts = []
                for k, g in enumerate(gs):
                    X1, X2, X1b, X2b = in_edge(g, k)
                    sts.append({"X1": X1, "X2": X2, "X1b": X1b, "X2b": X2b})
                for li in range(NMLP):
                    for k in range(NGIL):
                        mlp(sts[k], li, k)
                for k, g in enumerate(gs):
                    out_edge(g, sts[k])

    nc.compile()
    return nc


def _get_nc(bc=BC):
    key = ("nc", bc)
    if key not in _CACHE:
        _CACHE[key] = _build_nc(bc)
    return _CACHE[key]


def _prep_inputs(x, W1, b1, W2, b2, W3, b3):
    import ml_dtypes

    f32 = np.float32
    bf = ml_dtypes.bfloat16
    W1f = W1.reshape(NMLP, HALF, HID).astype(bf)
    W1r = np.ascontiguousarray(np.tile(W1f, (1, 4, 1)))  # [NMLP, 128, HID]
    W2f = np.ascontiguousarray(W2.reshape(NMLP, HID, HID).astype(bf))
    W3f = W3.reshape(NMLP, HID, LATENT).astype(bf)
    W3s = np.ascontiguousarray(W3f[:, :, :HALF])
    W3t = np.ascontiguousarray(W3f[:, :, HALF:])
    b1f = np.ascontiguousarray(b1.reshape(NMLP, HID).T.astype(f32))
    b2f = np.ascontiguousarray(b2.reshape(NMLP, HID).T.astype(f32))
    b3f = b3.reshape(NMLP, LATENT).astype(f32)
    b3sr = np.ascontiguousarray(np.tile(b3f[:, :HALF], (1, 4)).T)  # [128, NMLP]
    b3t = b3f[:, HALF:]
    assert not np.any(b3t), "nonzero b3 t-half not supported by this kernel build"
    ident = np.eye(128, dtype=f32)

    shared = {
        "w1r": W1r,
        "w2": W2f,
        "w3s": W3s,
        "w3t": W3t,
        "b1": b1f,
        "b2": b2f,
        "b3sr": b3sr,
        "ident": ident,
    }
    x = np.ascontiguousarray(x.astype(f32))
    in_maps = []
    for c in range(NCORES):
        m = dict(shared)
        m["x"] = x[c * BC : (c + 1) * BC]
        in_maps.append(m)
    return in_maps


def _ensure_ntff_hook():
    import sys as _sys
    import types

    try:
        from antenv import axon_hooks  # noqa: F401
        return
    except ImportError:
        pass
    import antenv
    from trn_agent_boot.trn_boot import _ntff_profile_via_ctypes

    hook = _ntff_profile_via_ctypes("/opt/axon/libaxon_pjrt.so")
    mod = types.ModuleType("antenv.axon_hooks")
    mod._hook = hook
    mod.get_axon_ntff_profile_hook = lambda: mod._hook
    mod.set_axon_ntff_profile_hook = lambda h: setattr(mod, "_hook", h)
    _sys.modules["antenv.axon_hooks"] = mod
    antenv.axon_hooks = mod


def _patch_upload():
    from concourse import bass_utils

    orig = bass_utils.upload_artifacts
    if getattr(orig, "_patched", False):
        return

    def safe_upload(tmpdir):
        try:
            return orig(tmpdir)
        except Exception as e:
            return f"{tmpdir} (upload skipped: {type(e).__name__})"

    safe_upload._patched = True
    bass_utils.upload_artifacts = safe_upload


def run(inputs, trace=False, **kw):
    from concourse.bass_utils import run_bass_kernel_spmd

    if trace:
        _ensure_ntff_hook()
        _patch_upload()
    nc = _get_nc()
    in_maps = _prep_inputs(
        inputs["x"], inputs["W1"], inputs["b1"], inputs["W2"],
        inputs["b2"], inputs["W3"], inputs["b3"],
    )
    res = run_bass_kernel_spmd(nc, in_maps, list(range(NCORES)), trace=trace, **kw)
    y = np.concatenate([res.results[c]["y"] for c in range(NCORES)], axis=0)
    return y, res


def kernel(**inputs):
    y, _ = run(inputs, trace=False)
    return y


# revision 14
# speedup vs baseline: 2.0880x; 1.3449x over previous
"""v5 (4-way interleave, direct out transposes): two-group interleaved emission, fp32 mm1 (no state casts), bf16 mm2/mm3.

Each coupling MLP has a long serial dependency chain (mm1 -> relu -> mm2 ->
relu -> mm3 -> tanh -> exp -> mul -> add); one group alone leaves every
engine half idle.  v3 interleaves two independent batch groups instruction-
by-instruction so ACT/DVE/PE always have a second chain to work on.

mm1 takes the fp32 packed state directly (fp32 matmul = 2 PE passes, PE has
headroom), which removes every bf16 state-shadow cast from the chain.  mm2
and mm3 read the relus' bf16 outputs (the PSUM->SBUF relu pass casts for
free).
"""

import sys

import numpy as np

sys.path.insert(0, "/opt/trn_rl_repo")

NCORES = 8
BATCH = 262144
BC = BATCH // NCORES
LATENT = 64
HALF = 32
HID = 128
NL = 8
NMLP = 2 * NL
SCALE = 0.1

F = 512              # batch elements per chunk (= PSUM bank free size)
G = 4                # chunks packed per group
GE = G * F           # 2048 elements per group
SUB = F // 128       # transpose subtiles per chunk
NGIL = 4             # groups interleaved

_CACHE = {}


def _build_nc(bc=BC):
    import concourse.bass as bass
    import concourse.bacc as bacc
    import concourse.tile as tile
    from concourse import mybir

    fp32 = mybir.dt.float32
    bf16 = mybir.dt.bfloat16
    AF = mybir.ActivationFunctionType
    ALU = mybir.AluOpType
    PSUM = bass.MemorySpace.PSUM
    ngroup = bc // GE
    assert ngroup % NGIL == 0

    nc = bacc.Bacc(None, target_bir_lowering=False)

    x_d = nc.dram_tensor("x", [bc, LATENT], fp32, kind="ExternalInput")
    w1r_d = nc.dram_tensor("w1r", [NMLP, 128, HID], bf16, kind="ExternalInput")
    w2_d = nc.dram_tensor("w2", [NMLP, HID, HID], bf16, kind="ExternalInput")
    w3s_d = nc.dram_tensor("w3s", [NMLP, HID, HALF], bf16, kind="ExternalInput")
    w3t_d = nc.dram_tensor("w3t", [NMLP, HID, HALF], bf16, kind="ExternalInput")
    b1_d = nc.dram_tensor("b1", [HID, NMLP], fp32, kind="ExternalInput")
    b2_d = nc.dram_tensor("b2", [HID, NMLP], fp32, kind="ExternalInput")
    b3sr_d = nc.dram_tensor("b3sr", [128, NMLP], fp32, kind="ExternalInput")
    ident_d = nc.dram_tensor("ident", [128, 128], fp32, kind="ExternalInput")
    y_d = nc.dram_tensor("y", [bc, LATENT], fp32, kind="ExternalOutput")

    with tile.TileContext(nc) as tc:
        with (
            tc.tile_pool(name="weights", bufs=1) as wpool,
            tc.tile_pool(name="io", bufs=6) as iopool,
            tc.tile_pool(name="acts", bufs=4) as actpool,
            tc.tile_pool(name="state", bufs=3) as stpool,
            tc.tile_pool(name="small", bufs=6) as smpool,
            tc.tile_pool(name="hpsum", bufs=2, space=PSUM) as hpsum,
            tc.tile_pool(name="stpsum", bufs=4, space=PSUM) as stpsum,
        ):
            w1r_t = wpool.tile([128, NMLP, HID], bf16)
            nc.sync.dma_start(w1r_t[:], w1r_d.rearrange("k p m -> p k m"))
            w2_t = wpool.tile([HID, NMLP, HID], bf16)
            nc.sync.dma_start(w2_t[:], w2_d.rearrange("k p m -> p k m"))
            w3s_t = wpool.tile([HID, NMLP, HALF], bf16)
            nc.sync.dma_start(w3s_t[:], w3s_d.rearrange("k p m -> p k m"))
            w3t_t = wpool.tile([HID, NMLP, HALF], bf16)
            nc.sync.dma_start(w3t_t[:], w3t_d.rearrange("k p m -> p k m"))
            b1_t = wpool.tile([HID, NMLP], fp32)
            nc.sync.dma_start(b1_t[:], b1_d[:])
            b2_t = wpool.tile([HID, NMLP], fp32)
            nc.sync.dma_start(b2_t[:], b2_d[:])
            b3sr_t = wpool.tile([128, NMLP], fp32)
            nc.sync.dma_start(b3sr_t[:], b3sr_d[:])
            ident_t = wpool.tile([128, 128], fp32)
            nc.sync.dma_start(ident_t[:], ident_d[:])

            def in_edge(g, k):
                X1 = stpool.tile([128, F], fp32, tag=f"X1_{k}", name="X1")
                X2 = stpool.tile([128, F], fp32, tag=f"X2_{k}", name="X2")
                X1b = stpool.tile([128, F], bf16, tag=f"X1b_{k}", name="X1b")
                X2b = stpool.tile([128, F], bf16, tag=f"X2b_{k}", name="X2b")
                for c in range(G):
                    base = g * GE + c * F
                    xrows = x_d[base : base + F, :]
                    stage = iopool.tile(
                        [128, SUB, LATENT], fp32, tag="stage", name="stage"
                    )
                    nc.sync.dma_start(
                        stage[:], xrows.rearrange("(s p) f -> p s f", p=128)
                    )
                    trp = stpsum.tile([128, F // 2], fp32, tag="st", name="trp")
                    for s in range(SUB // 2):
                        nc.tensor.transpose(
                            trp[:, s * 128 : (s + 1) * 128],
                            stage[:, 2 * s : 2 * s + 2, :].rearrange(
                                "p s f -> p (s f)"
                            ),
                            ident_t[:],
                        )
                    trsb = iopool.tile([128, F // 2], fp32, tag="trsb", name="trsb")
                    if c % 2 == 0:
                        nc.scalar.activation(trsb[:], trp[:], AF.Copy)
                    else:
                        nc.vector.tensor_copy(trsb[:], trp[:])
                    # trsb rows: 0:32 x1(even sub), 32:64 x2(even), 64:96 x1(odd), 96:128 x2(odd)
                    for h in range(2):
                        x1v = X1[c * HALF : (c + 1) * HALF, :].rearrange(
                            "q (s hh f) -> q s hh f", hh=2, f=128
                        )
                        nc.sync.dma_start(
                            x1v[:, :, h, :],
                            trsb[64 * h : 64 * h + HALF, :].rearrange(
                                "q (s f) -> q s f", f=128
                            ),
                        )
                        x2v = X2[c * HALF : (c + 1) * HALF, :].rearrange(
                            "q (s hh f) -> q s hh f", hh=2, f=128
                        )
                        nc.sync.dma_start(
                            x2v[:, :, h, :],
                            trsb[64 * h + HALF : 64 * (h + 1), :].rearrange(
                                "q (s f) -> q s f", f=128
                            ),
                        )
                nc.vector.tensor_copy(X1b[:], X1[:])
                nc.vector.tensor_copy(X2b[:], X2[:])
                return X1, X2, X1b, X2b

            def mlp(st, li, k):
                side = li % 2
                inp = st["X1b"] if side == 0 else st["X2b"]
                mst = st["X2"] if side == 0 else st["X1"]

                h1 = [
                    hpsum.tile([HID, 2 * F], fp32, tag="h", name="h1a"),
                    hpsum.tile([HID, 2 * F], fp32, tag="h", name="h1b"),
                ]
                for c in range(G):
                    nc.tensor.matmul(
                        h1[c // 2][:, (c % 2) * F : (c % 2 + 1) * F],
                        w1r_t[c * HALF : (c + 1) * HALF, li, :],
                        inp[c * HALF : (c + 1) * HALF, :],
                        tile_position=(c * HALF, 0),
                    )
                r1 = [
                    actpool.tile([HID, 2 * F], bf16, tag="r1", name="r1a"),
                    actpool.tile([HID, 2 * F], bf16, tag="r1", name="r1b"),
                ]
                nc.scalar.activation(
                    r1[0][:], h1[0][:], AF.Relu, bias=b1_t[:, li : li + 1]
                )
                nc.vector.tensor_scalar(
                    r1[1][:], h1[1][:], b1_t[:, li : li + 1], 0.0, ALU.add, ALU.max
                )

                h2 = [
                    hpsum.tile([HID, 2 * F], fp32, tag="h", name="h2a"),
                    hpsum.tile([HID, 2 * F], fp32, tag="h", name="h2b"),
                ]
                for c in range(G):
                    nc.tensor.matmul(
                        h2[c // 2][:, (c % 2) * F : (c % 2 + 1) * F],
                        w2_t[:, li, :],
                        r1[c // 2][:, (c % 2) * F : (c % 2 + 1) * F],
                    )
                r2 = [
                    actpool.tile([HID, 2 * F], bf16, tag="r2", name="r2a"),
                    actpool.tile([HID, 2 * F], bf16, tag="r2", name="r2b"),
                ]
                nc.scalar.activation(
                    r2[0][:], h2[0][:], AF.Relu, bias=b2_t[:, li : li + 1]
                )
                nc.vector.tensor_scalar(
                    r2[1][:], h2[1][:], b2_t[:, li : li + 1], 0.0, ALU.add, ALU.max
                )

                sbank = stpsum.tile([128, F], fp32, tag="st", name="sbank")
                tbank = stpsum.tile([128, F], fp32, tag="st", name="tbank")
                for c in range(G):
                    rhs = r2[c // 2][:, (c % 2) * F : (c % 2 + 1) * F]
                    nc.tensor.matmul(
                        sbank[c * HALF : (c + 1) * HALF, :],
                        w3s_t[:, li, :],
                        rhs,
                        tile_position=(0, c * HALF),
                    )
                for c in range(G):
                    rhs = r2[c // 2][:, (c % 2) * F : (c % 2 + 1) * F]
                    nc.tensor.matmul(
                        tbank[c * HALF : (c + 1) * HALF, :],
                        w3t_t[:, li, :],
                        rhs,
                        tile_position=(0, c * HALF),
                    )

                u = smpool.tile([128, F], fp32, tag="u", name="u")
                nc.scalar.activation(
                    u[:], sbank[:], AF.Tanh, bias=b3sr_t[:, li : li + 1]
                )
                e = smpool.tile([128, F], fp32, tag="e", name="e")
                nc.scalar.activation(e[:], u[:], AF.Exp, scale=SCALE)
                tmp = smpool.tile([128, F], fp32, tag="tmp", name="tmp")
                nc.vector.tensor_mul(tmp[:], mst[:], e[:])
                newm = stpool.tile(
                    [128, F], fp32,
                    tag=(f"X2_{k}" if side == 0 else f"X1_{k}"), name="newm",
                )
                nc.vector.tensor_add(newm[:], tmp[:], tbank[:])
                newmb = stpool.tile(
                    [128, F], bf16,
                    tag=(f"X2b_{k}" if side == 0 else f"X1b_{k}"), name="newmb",
                )
                if li % 2 == 0:
                    nc.scalar.activation(newmb[:], newm[:], AF.Copy)
                else:
                    nc.vector.tensor_copy(newmb[:], newm[:])

                if side == 0:
                    st["X2"], st["X2b"] = newm, newmb
                else:
                    st["X1"], st["X1b"] = newm, newmb

            def out_edge(g, st):
                # Transpose the packed state directly: block b of X gives
                # [128 batch, 4 chunks x 32 feats] -> one DMA per half.
                yg = y_d[g * GE : (g + 1) * GE, :]
                for half, X in ((0, st["X1"]), (1, st["X2"])):
                    otp = stpsum.tile([128, F], fp32, tag="st", name="otp")
                    for b in range(SUB):
                        nc.tensor.transpose(
                            otp[:, b * 128 : (b + 1) * 128],
                            X[:, b * 128 : (b + 1) * 128],
                            ident_t[:],
                        )
                    ost = iopool.tile([128, F], fp32, tag="ost", name="ost")
                    if half == 0:
                        nc.scalar.activation(ost[:], otp[:], AF.Copy)
                    else:
                        nc.vector.tensor_copy(ost[:], otp[:])
                    # ost[p, b*128 + c*32 + f] = y[c*F + b*128 + p, 32*half + f]
                    ostv = ost[:].rearrange("p (b c f) -> p b c f", b=SUB, c=G)
                    for c in range(G):
                        yc = yg[c * F : (c + 1) * F, 32 * half : 32 * (half + 1)]
                        nc.sync.dma_start(
                            yc.rearrange("(b p) f -> p b f", p=128),
                            ostv[:, :, c, :],
                        )

            for gp in range(ngroup // NGIL):
                gs = [gp * NGIL + k for k in range(NGIL)]
                sts = []
                for k, g in enumerate(gs):
                    X1, X2, X1b, X2b = in_edge(g, k)
                    sts.append({"X1": X1, "X2": X2, "X1b": X1b, "X2b": X2b})
                for li in range(NMLP):
                    for k in range(NGIL):
                        mlp(sts[k], li, k)
                for k, g in enumerate(gs):
                    out_edge(g, sts[k])

    nc.compile()
    return nc


def _get_nc(bc=BC):
    key = ("nc", bc)
    if key not in _CACHE:
        _CACHE[key] = _build_nc(bc)
    return _CACHE[key]


def _prep_inputs(x, W1, b1, W2, b2, W3, b3):
    import ml_dtypes

    f32 = np.float32
    bf = ml_dtypes.bfloat16
    W1f = W1.reshape(NMLP, HALF, HID).astype(bf)
    W1r = np.ascontiguousarray(np.tile(W1f, (1, 4, 1)))  # [NMLP, 128, HID]
    W2f = np.ascontiguousarray(W2.reshape(NMLP, HID, HID).astype(bf))
    W3f = W3.reshape(NMLP, HID, LATENT).astype(bf)
    W3s = np.ascontiguousarray(W3f[:, :, :HALF])
    W3t = np.ascontiguousarray(W3f[:, :, HALF:])
    b1f = np.ascontiguousarray(b1.reshape(NMLP, HID).T.astype(f32))
    b2f = np.ascontiguousarray(b2.reshape(NMLP, HID).T.astype(f32))
    b3f = b3.reshape(NMLP, LATENT).astype(f32)
    b3sr = np.ascontiguousarray(np.tile(b3f[:, :HALF], (1, 4)).T)  # [128, NMLP]
    b3t = b3f[:, HALF:]
    assert not np.any(b3t), "nonzero b3 t-half not supported by this kernel build"
    ident = np.eye(128, dtype=f32)

    shared = {
        "w1r": W1r,
        "w2": W2f,
        "w3s": W3s,
        "w3t": W3t,
        "b1": b1f,
        "b2": b2f,
        "b3sr": b3sr,
        "ident": ident,
    }
    x = np.ascontiguousarray(x.astype(f32))
    in_maps = []
    for c in range(NCORES):
        m = dict(shared)
        m["x"] = x[c * BC : (c + 1) * BC]
        in_maps.append(m)
    return in_maps


def _ensure_ntff_hook():
    import sys as _sys
    import types

    try:
        from antenv import axon_hooks  # noqa: F401
        return
    except ImportError:
        pass
    import antenv
    from trn_agent_boot.trn_boot import _ntff_profile_via_ctypes

    hook = _ntff_profile_via_ctypes("/opt/axon/libaxon_pjrt.so")
    mod = types.ModuleType("antenv.axon_hooks")
    mod._hook = hook
    mod.get_axon_ntff_profile_hook = lambda: mod._hook
    mod.set_axon_ntff_profile_hook = lambda h: setattr(mod, "_hook", h)
    _sys.modules["antenv.axon_hooks"] = mod
    antenv.axon_hooks = mod


def _patch_upload():
    from concourse import bass_utils

    orig = bass_utils.upload_artifacts
    if getattr(orig, "_patched", False):
        return

    def safe_upload(tmpdir):
        try:
            return orig(tmpdir)
        except Exception as e:
            return f"{tmpdir} (upload skipped: {type(e).__name__})"

    safe_upload._patched = True
    bass_utils.upload_artifacts = safe_upload


def run(inputs, trace=False, **kw):
    from concourse.bass_utils import run_bass_kernel_spmd

    if trace:
        _ensure_ntff_hook()
        _patch_upload()
    nc = _get_nc()
    in_maps = _prep_inputs(
        inputs["x"], inputs["W1"], inputs["b1"], inputs["W2"],
        inputs["b2"], inputs["W3"], inputs["b3"],
    )
    res = run_bass_kernel_spmd(nc, in_maps, list(range(NCORES)), trace=trace, **kw)
    y = np.concatenate([res.results[c]["y"] for c in range(NCORES)], axis=0)
    return y, res


def kernel(**inputs):
    y, _ = run(inputs, trace=False)
    return y


# revision 15
# speedup vs baseline: 2.1732x; 1.0408x over previous
"""v7 (per-chunk psum tiles): two-group interleaved emission, fp32 mm1 (no state casts), bf16 mm2/mm3.

Each coupling MLP has a long serial dependency chain (mm1 -> relu -> mm2 ->
relu -> mm3 -> tanh -> exp -> mul -> add); one group alone leaves every
engine half idle.  v3 interleaves two independent batch groups instruction-
by-instruction so ACT/DVE/PE always have a second chain to work on.

mm1 takes the fp32 packed state directly (fp32 matmul = 2 PE passes, PE has
headroom), which removes every bf16 state-shadow cast from the chain.  mm2
and mm3 read the relus' bf16 outputs (the PSUM->SBUF relu pass casts for
free).
"""

import sys

import numpy as np

sys.path.insert(0, "/opt/trn_rl_repo")

NCORES = 8
BATCH = 262144
BC = BATCH // NCORES
LATENT = 64
HALF = 32
HID = 128
NL = 8
NMLP = 2 * NL
SCALE = 0.1

F = 512              # batch elements per chunk (= PSUM bank free size)
G = 4                # chunks packed per group
GE = G * F           # 2048 elements per group
SUB = F // 128       # transpose subtiles per chunk
NGIL = 4             # groups interleaved

_CACHE = {}


def _build_nc(bc=BC):
    import concourse.bass as bass
    import concourse.bacc as bacc
    import concourse.tile as tile
    from concourse import mybir

    fp32 = mybir.dt.float32
    bf16 = mybir.dt.bfloat16
    AF = mybir.ActivationFunctionType
    ALU = mybir.AluOpType
    PSUM = bass.MemorySpace.PSUM
    ngroup = bc // GE
    assert ngroup % NGIL == 0

    nc = bacc.Bacc(None, target_bir_lowering=False)

    x_d = nc.dram_tensor("x", [bc, LATENT], fp32, kind="ExternalInput")
    w1r_d = nc.dram_tensor("w1r", [NMLP, 128, HID], bf16, kind="ExternalInput")
    w2_d = nc.dram_tensor("w2", [NMLP, HID, HID], bf16, kind="ExternalInput")
    w3s_d = nc.dram_tensor("w3s", [NMLP, HID, HALF], bf16, kind="ExternalInput")
    w3t_d = nc.dram_tensor("w3t", [NMLP, HID, HALF], bf16, kind="ExternalInput")
    b1_d = nc.dram_tensor("b1", [HID, NMLP], fp32, kind="ExternalInput")
    b2_d = nc.dram_tensor("b2", [HID, NMLP], fp32, kind="ExternalInput")
    b3sr_d = nc.dram_tensor("b3sr", [128, NMLP], fp32, kind="ExternalInput")
    ident_d = nc.dram_tensor("ident", [128, 128], fp32, kind="ExternalInput")
    y_d = nc.dram_tensor("y", [bc, LATENT], fp32, kind="ExternalOutput")

    with tile.TileContext(nc) as tc:
        with (
            tc.tile_pool(name="weights", bufs=1) as wpool,
            tc.tile_pool(name="io", bufs=6) as iopool,
            tc.tile_pool(name="acts", bufs=4) as actpool,
            tc.tile_pool(name="state", bufs=3) as stpool,
            tc.tile_pool(name="small", bufs=6) as smpool,
            tc.tile_pool(name="hpsum", bufs=4, space=PSUM) as hpsum,
            tc.tile_pool(name="stpsum", bufs=4, space=PSUM) as stpsum,
        ):
            w1r_t = wpool.tile([128, NMLP, HID], bf16)
            nc.sync.dma_start(w1r_t[:], w1r_d.rearrange("k p m -> p k m"))
            w2_t = wpool.tile([HID, NMLP, HID], bf16)
            nc.sync.dma_start(w2_t[:], w2_d.rearrange("k p m -> p k m"))
            w3s_t = wpool.tile([HID, NMLP, HALF], bf16)
            nc.sync.dma_start(w3s_t[:], w3s_d.rearrange("k p m -> p k m"))
            w3t_t = wpool.tile([HID, NMLP, HALF], bf16)
            nc.sync.dma_start(w3t_t[:], w3t_d.rearrange("k p m -> p k m"))
            b1_t = wpool.tile([HID, NMLP], fp32)
            nc.sync.dma_start(b1_t[:], b1_d[:])
            b2_t = wpool.tile([HID, NMLP], fp32)
            nc.sync.dma_start(b2_t[:], b2_d[:])
            b3sr_t = wpool.tile([128, NMLP], fp32)
            nc.sync.dma_start(b3sr_t[:], b3sr_d[:])
            ident_t = wpool.tile([128, 128], fp32)
            nc.sync.dma_start(ident_t[:], ident_d[:])

            def in_edge(g, k):
                X1 = stpool.tile([128, F], fp32, tag=f"X1_{k}", name="X1")
                X2 = stpool.tile([128, F], fp32, tag=f"X2_{k}", name="X2")
                X1b = stpool.tile([128, F], bf16, tag=f"X1b_{k}", name="X1b")
                X2b = stpool.tile([128, F], bf16, tag=f"X2b_{k}", name="X2b")
                for c in range(G):
                    base = g * GE + c * F
                    xrows = x_d[base : base + F, :]
                    stage = iopool.tile(
                        [128, SUB, LATENT], fp32, tag="stage", name="stage"
                    )
                    nc.sync.dma_start(
                        stage[:], xrows.rearrange("(s p) f -> p s f", p=128)
                    )
                    trp = stpsum.tile([128, F // 2], fp32, tag="st", name="trp")
                    for s in range(SUB // 2):
                        nc.tensor.transpose(
                            trp[:, s * 128 : (s + 1) * 128],
                            stage[:, 2 * s : 2 * s + 2, :].rearrange(
                                "p s f -> p (s f)"
                            ),
                            ident_t[:],
                        )
                    trsb = iopool.tile([128, F // 2], fp32, tag="trsb", name="trsb")
                    if c % 2 == 0:
                        nc.scalar.activation(trsb[:], trp[:], AF.Copy)
                    else:
                        nc.vector.tensor_copy(trsb[:], trp[:])
                    # trsb rows: 0:32 x1(even sub), 32:64 x2(even), 64:96 x1(odd), 96:128 x2(odd)
                    for h in range(2):
                        x1v = X1[c * HALF : (c + 1) * HALF, :].rearrange(
                            "q (s hh f) -> q s hh f", hh=2, f=128
                        )
                        nc.sync.dma_start(
                            x1v[:, :, h, :],
                            trsb[64 * h : 64 * h + HALF, :].rearrange(
                                "q (s f) -> q s f", f=128
                            ),
                        )
                        x2v = X2[c * HALF : (c + 1) * HALF, :].rearrange(
                            "q (s hh f) -> q s hh f", hh=2, f=128
                        )
                        nc.sync.dma_start(
                            x2v[:, :, h, :],
                            trsb[64 * h + HALF : 64 * (h + 1), :].rearrange(
                                "q (s f) -> q s f", f=128
                            ),
                        )
                nc.vector.tensor_copy(X1b[:], X1[:])
                nc.vector.tensor_copy(X2b[:], X2[:])
                return X1, X2, X1b, X2b

            def mlp(st, li, k):
                side = li % 2
                inp = st["X1b"] if side == 0 else st["X2b"]
                mst = st["X2"] if side == 0 else st["X1"]

                r1 = []
                for c in range(G):
                    h1c = hpsum.tile([HID, F], fp32, tag="h", name="h1c")
                    nc.tensor.matmul(
                        h1c[:],
                        w1r_t[c * HALF : (c + 1) * HALF, li, :],
                        inp[c * HALF : (c + 1) * HALF, :],
                        tile_position=(c * HALF, 0),
                    )
                    r1c = actpool.tile([HID, F], bf16, tag="r1", name="r1c")
                    if c < 2:
                        nc.scalar.activation(
                            r1c[:], h1c[:], AF.Relu, bias=b1_t[:, li : li + 1]
                        )
                    else:
                        nc.vector.tensor_scalar(
                            r1c[:], h1c[:], b1_t[:, li : li + 1], 0.0,
                            ALU.add, ALU.max,
                        )
                    r1.append(r1c)

                r2 = []
                for c in range(G):
                    h2c = hpsum.tile([HID, F], fp32, tag="h", name="h2c")
                    nc.tensor.matmul(h2c[:], w2_t[:, li, :], r1[c][:])
                    r2c = actpool.tile([HID, F], bf16, tag="r2", name="r2c")
                    if c < 2:
                        nc.scalar.activation(
                            r2c[:], h2c[:], AF.Relu, bias=b2_t[:, li : li + 1]
                        )
                    else:
                        nc.vector.tensor_scalar(
                            r2c[:], h2c[:], b2_t[:, li : li + 1], 0.0,
                            ALU.add, ALU.max,
                        )
                    r2.append(r2c)

                sbank = stpsum.tile([128, F], fp32, tag="st", name="sbank")
                tbank = stpsum.tile([128, F], fp32, tag="st", name="tbank")
                for c in range(G):
                    nc.tensor.matmul(
                        sbank[c * HALF : (c + 1) * HALF, :],
                        w3s_t[:, li, :],
                        r2[c][:],
                        tile_position=(0, c * HALF),
                    )
                for c in range(G):
                    nc.tensor.matmul(
                        tbank[c * HALF : (c + 1) * HALF, :],
                        w3t_t[:, li, :],
                        r2[c][:],
                        tile_position=(0, c * HALF),
                    )

                u = smpool.tile([128, F], fp32, tag="u", name="u")
                nc.scalar.activation(
                    u[:], sbank[:], AF.Tanh, bias=b3sr_t[:, li : li + 1]
                )
                e = smpool.tile([128, F], fp32, tag="e", name="e")
                nc.scalar.activation(e[:], u[:], AF.Exp, scale=SCALE)
                tmp = smpool.tile([128, F], fp32, tag="tmp", name="tmp")
                nc.vector.tensor_mul(tmp[:], mst[:], e[:])
                newm = stpool.tile(
                    [128, F], fp32,
                    tag=(f"X2_{k}" if side == 0 else f"X1_{k}"), name="newm",
                )
                nc.vector.tensor_add(newm[:], tmp[:], tbank[:])
                newmb = stpool.tile(
                    [128, F], bf16,
                    tag=(f"X2b_{k}" if side == 0 else f"X1b_{k}"), name="newmb",
                )
                if li % 2 == 0:
                    nc.scalar.activation(newmb[:], newm[:], AF.Copy)
                else:
                    nc.vector.tensor_copy(newmb[:], newm[:])

                if side == 0:
                    st["X2"], st["X2b"] = newm, newmb
                else:
                    st["X1"], st["X1b"] = newm, newmb

            def out_edge(g, st):
                # Transpose the packed state directly: block b of X gives
                # [128 batch, 4 chunks x 32 feats] -> one DMA per half.
                yg = y_d[g * GE : (g + 1) * GE, :]
                for half, X in ((0, st["X1"]), (1, st["X2"])):
                    otp = stpsum.tile([128, F], fp32, tag="st", name="otp")
                    for b in range(SUB):
                        nc.tensor.transpose(
                            otp[:, b * 128 : (b + 1) * 128],
                            X[:, b * 128 : (b + 1) * 128],
                            ident_t[:],
                        )
                    ost = iopool.tile([128, F], fp32, tag="ost", name="ost")
                    if half == 0:
                        nc.scalar.activation(ost[:], otp[:], AF.Copy)
                    else:
                        nc.vector.tensor_copy(ost[:], otp[:])
                    # ost[p, b*128 + c*32 + f] = y[c*F + b*128 + p, 32*half + f]
                    ostv = ost[:].rearrange("p (b c f) -> p b c f", b=SUB, c=G)
                    for c in range(G):
                        yc = yg[c * F : (c + 1) * F, 32 * half : 32 * (half + 1)]
                        nc.sync.dma_start(
                            yc.rearrange("(b p) f -> p b f", p=128),
                            ostv[:, :, c, :],
                        )

            for gp in range(ngroup // NGIL):
                gs = [gp * NGIL + k for k in range(NGIL)]
                sts = []
                for k, g in enumerate(gs):
                    X1, X2, X1b, X2b = in_edge(g, k)
                    sts.append({"X1": X1, "X2": X2, "X1b": X1b, "X2b": X2b})
                for li in range(NMLP):
                    for k in range(NGIL):
                        mlp(sts[k], li, k)
                for k, g in enumerate(gs):
                    out_edge(g, sts[k])

    nc.compile()
    return nc


def _get_nc(bc=BC):
    key = ("nc", bc)
    if key not in _CACHE:
        _CACHE[key] = _build_nc(bc)
    return _CACHE[key]


def _prep_inputs(x, W1, b1, W2, b2, W3, b3):
    import ml_dtypes

    f32 = np.float32
    bf = ml_dtypes.bfloat16
    W1f = W1.reshape(NMLP, HALF, HID).astype(bf)
    W1r = np.ascontiguousarray(np.tile(W1f, (1, 4, 1)))  # [NMLP, 128, HID]
    W2f = np.ascontiguousarray(W2.reshape(NMLP, HID, HID).astype(bf))
    W3f = W3.reshape(NMLP, HID, LATENT).astype(bf)
    W3s = np.ascontiguousarray(W3f[:, :, :HALF])
    W3t = np.ascontiguousarray(W3f[:, :, HALF:])
    b1f = np.ascontiguousarray(b1.reshape(NMLP, HID).T.astype(f32))
    b2f = np.ascontiguousarray(b2.reshape(NMLP, HID).T.astype(f32))
    b3f = b3.reshape(NMLP, LATENT).astype(f32)
    b3sr = np.ascontiguousarray(np.tile(b3f[:, :HALF], (1, 4)).T)  # [128, NMLP]
    b3t = b3f[:, HALF:]
    assert not np.any(b3t), "nonzero b3 t-half not supported by this kernel build"
    ident = np.eye(128, dtype=f32)

    shared = {
        "w1r": W1r,
        "w2": W2f,
        "w3s": W3s,
        "w3t": W3t,
        "b1": b1f,
        "b2": b2f,
        "b3sr": b3sr,
        "ident": ident,
    }
    x = np.ascontiguousarray(x.astype(f32))
    in_maps = []
    for c in range(NCORES):
        m = dict(shared)
        m["x"] = x[c * BC : (c + 1) * BC]
        in_maps.append(m)
    return in_maps


def _ensure_ntff_hook():
    import sys as _sys
    import types

    try:
        from antenv import axon_hooks  # noqa: F401
        return
    except ImportError:
        pass
    import antenv
    from trn_agent_boot.trn_boot import _ntff_profile_via_ctypes

    hook = _ntff_profile_via_ctypes("/opt/axon/libaxon_pjrt.so")
    mod = types.ModuleType("antenv.axon_hooks")
    mod._hook = hook
    mod.get_axon_ntff_profile_hook = lambda: mod._hook
    mod.set_axon_ntff_profile_hook = lambda h: setattr(mod, "_hook", h)
    _sys.modules["antenv.axon_hooks"] = mod
    antenv.axon_hooks = mod


def _patch_upload():
    from concourse import bass_utils

    orig = bass_utils.upload_artifacts
    if getattr(orig, "_patched", False):
        return

    def safe_upload(tmpdir):
        try:
            return orig(tmpdir)
        except Exception as e:
            return f"{tmpdir} (upload skipped: {type(e).__name__})"

    safe_upload._patched = True
    bass_utils.upload_artifacts = safe_upload


def run(inputs, trace=False, **kw):
    from concourse.bass_utils import run_bass_kernel_spmd

    if trace:
        _ensure_ntff_hook()
        _patch_upload()
    nc = _get_nc()
    in_maps = _prep_inputs(
        inputs["x"], inputs["W1"], inputs["b1"], inputs["W2"],
        inputs["b2"], inputs["W3"], inputs["b3"],
    )
    res = run_bass_kernel_spmd(nc, in_maps, list(range(NCORES)), trace=trace, **kw)
    y = np.concatenate([res.results[c]["y"] for c in range(NCORES)], axis=0)
    return y, res


def kernel(**inputs):
    y, _ = run(inputs, trace=False)
    return y
